# revision 1
# baseline (speedup 1.0000x reference)
"""Trainium2 Bass kernel for nn_GroupedLossWithIndexMap.

Reference computation (per batch item b, N=65536 rows, C_old=128, C_new=16):
    probs   = softmax(inputs[b], axis=-1)            # [N, 128]
    grouped = probs @ GROUP_MAT                      # [N, 16] (8 contiguous cols per group)
    avg     = mean(grouped, axis=0)                  # [16]
    loss_b  = KL(softmax(targets[b]/100) || softmax(avg)) / 16
    out     = mean_b(loss_b)

Key identity: grouping+mean commute, so each core only needs
    colsum[c] = sum_n exp(x[n,c]) / rowsum[n]        # [128]
and the rest is trivial scalar math done on host.

Device kernel (per core, one batch item, data parallel over 8 cores):
  - 16 groups of 4096 rows; each group is one contiguous 2 MB DMA into a
    [128, 32*128] SBUF tile (partition p holds 32 consecutive rows).
  - ACT: exp (f32 -> bf16).
  - DVE: row sums (reduce innermost of [128,32,128]), reciprocal, bf16 copy.
  - PE : 32 tiny matmuls per group: psum[1,128] += r_k^T @ exp_k
         (stationary = reciprocal column [128,1], moving = exp chunk [128,128]),
         one accumulation chain across all 512 chunks.
  - out: colsum [1,128] f32 -> DRAM.
"""

import numpy as np

B = 8
N = 65536
C = 128
G = 16
P = 128
K = 32          # rows per partition per group tile
NG = N // (P * K)   # 16 groups
EPS = 1e-8

_compiled = None


def _patch_tile_epilogue(tile):
    """Replace TileContext's end-of-kernel drain+barrier with a lighter one:
    the sync.drain already waits on the global completion clock, so the two
    all-engine barriers around the semaphore clears only need sequencer-level
    (sem_only) sync — the per-engine InstDrains they normally emit cost ~9us."""
    if getattr(tile.TileContext, "_fast_epilogue", False):
        return
    from concourse.vector_clock import ScopedClock

    def _drain_and_barrier(self, tick_clock, wait_clock):
        drain_inst = self.nc.sync.drain()
        wait_clock.add_sem_waits(
            drain_inst.ins, ScopedClock({None: tick_clock.global_clock})
        )
        self.nc.all_engine_barrier(sem_only=True)
        popped = self.nc._tile_sem_poison_stack.pop()
        assert popped is self._sem_poison
        self.nc.clear_and_free_semaphores(list(self.sems.allocated().values()))
        self.nc.all_engine_barrier(sem_only=True)

    tile.TileContext._drain_and_barrier = _drain_and_barrier
    tile.TileContext._fast_epilogue = True


def _build(ng: int = NG):
    import concourse.bacc as bacc
    import concourse.bass as bass
    import concourse.tile as tile
    from concourse import mybir

    _patch_tile_epilogue(tile)

    f32 = mybir.dt.float32
    bf16 = mybir.dt.bfloat16

    n = P * K * ng

    nc = bacc.Bacc(
        "TRN2",
        target_bir_lowering=False,
        debug=False,
        num_devices=B,
    )

    x = nc.dram_tensor("x", [n, C], f32, kind="ExternalInput")
    colsum = nc.dram_tensor("colsum", [4, 4 * C], f32, kind="ExternalOutput")

    # Group schedule: bulk groups of K rows/partition, then a gradually
    # tapered tail so each pipeline stage's backlog drains with the data;
    # the post-last-DMA drain is then only the small last group's chain.
    if ng == NG:
        specs = [K] * 14 + [16, 16, 8, 8, 8, 4, 4]
    else:
        specs = [K] * ng
    assert sum(specs) == ng * K

    MB = 4  # matmul chunk block: MB chunks of 128 rows per matmul instruction

    # DRAM row offset (in units of P rows) per group
    with tile.TileContext(nc) as tc:
        with (
            tc.tile_pool(name="xin", bufs=4) as xpool,
            tc.tile_pool(name="exp", bufs=6) as epool,
            tc.tile_pool(name="half", bufs=4) as hpool,
            tc.tile_pool(name="small", bufs=6) as spool,
            tc.tile_pool(name="out", bufs=1) as opool,
            tc.tile_pool(name="psum", bufs=1, space="PSUM") as ppool,
        ):
            ps = ppool.tile([MB, MB * C], f32)
            row0 = 0
            nmm = sum((kk + MB - 1) // MB for kk in specs)
            mmi = 0
            for g, kk in enumerate(specs):
                # partition p holds rows row0 + p*kk + [0, kk)
                src = (
                    x.ap()[row0 : row0 + P * kk, :]
                    .rearrange("(p k) c -> p (k c)", p=P, k=kk)
                )
                xt = xpool.tile([P, kk * C], f32, tag="x")
                nc.sync.dma_start(out=xt[:], in_=src)

                et = epool.tile([P, kk * C], bf16, tag="e")
                nc.scalar.activation(et[:], xt[:], mybir.ActivationFunctionType.Exp)

                e3 = et[:].rearrange("p (k c) -> p k c", c=C)
                st = spool.tile([P, kk], f32, tag="s")
                if kk <= 8:
                    # tiny tail group: single reduce beats 3 instructions
                    nc.vector.reduce_sum(st[:], e3, axis=mybir.AxisListType.X)
                else:
                    at = hpool.tile([P, kk * 64], bf16, tag="a")
                    a3 = at[:].rearrange("p (k c) -> p k c", c=64)
                    nc.vector.tensor_add(a3, e3[:, :, 0:64], e3[:, :, 64:128])
                    bt = hpool.tile([P, kk * 32], bf16, tag="b")
                    b3 = bt[:].rearrange("p (k c) -> p k c", c=32)
                    nc.vector.tensor_add(b3, a3[:, :, 0:32], a3[:, :, 32:64])
                    nc.vector.reduce_sum(st[:], b3, axis=mybir.AxisListType.X)
                rb = spool.tile([P, kk], bf16, tag="rb")
                with nc.allow_low_precision("bf16 reciprocal weights"):
                    nc.vector.reciprocal(rb[:], st[:])

                for k0 in range(0, kk, MB):
                    m = min(MB, kk - k0)
                    nc.tensor.matmul(
                        ps[0:m, 0 : m * C],
                        rb[:, k0 : k0 + m],
                        et[:, k0 * C : (k0 + m) * C],
                        start=(mmi == 0),
                        stop=(mmi == nmm - 1),
                    )
                    mmi += 1
                row0 += P * kk
            assert mmi == nmm

            # Diagonal blocks of ps hold the real partial colsums; off-diagonal
            # blocks are accumulation garbage. Ship the whole tile and let the
            # host pick the diagonal (engines can't start at partition>0, and
            # DMA can't read PSUM directly).
            ot = opool.tile([MB, MB * C], f32)
            nc.vector.tensor_copy(ot[:], ps[:])
            nc.sync.dma_start(out=colsum[:], in_=ot[:])

    nc.compile()
    return nc


def _get_compiled():
    global _compiled
    if _compiled is None:
        _compiled = _build()
    return _compiled


def _run_device(inputs: np.ndarray, trace: bool = False, **kwargs):
    from concourse.bass_utils import run_bass_kernel_spmd

    nc = _get_compiled()
    in_maps = [
        {"x": np.ascontiguousarray(inputs[i], dtype=np.float32)} for i in range(B)
    ]
    res = run_bass_kernel_spmd(nc, in_maps, list(range(B)), trace=trace, **kwargs)
    colsums = np.stack(
        [
            np.asarray(res.results[i]["colsum"], dtype=np.float64)
            .reshape(4, 4, C)[np.arange(4), np.arange(4)]
            .sum(axis=0)
            for i in range(B)
        ]
    )  # [B, 128]
    return colsums, res


def _finish_host(colsums: np.ndarray, targets: np.ndarray) -> np.ndarray:
    # colsums: [B, 128] float; targets: [B, 16]
    cs = colsums.astype(np.float64)
    avg = cs.reshape(B, G, C // G).sum(axis=-1) / N          # [B, 16]
    # softmax(avg)
    a = avg - avg.max(axis=-1, keepdims=True)
    p = np.exp(a)
    p /= p.sum(axis=-1, keepdims=True)
    # softmax(targets / 100)
    t = targets.astype(np.float64) / 100.0
    t = t - t.max(axis=-1, keepdims=True)
    t = np.exp(t)
    t /= t.sum(axis=-1, keepdims=True)
    log_p = np.log(p + EPS)
    kl = (t * (np.log(t) - log_p)).sum(axis=-1) / G          # [B]
    return np.float32(kl.mean())


def kernel(inputs: np.ndarray, targets: np.ndarray) -> np.ndarray:
    colsums, _ = _run_device(np.asarray(inputs))
    return _finish_host(colsums, np.asarray(targets))

